# revision 3
# baseline (speedup 1.0000x reference)
# Banked (MoE top-2) feedforward on 8 TRN2 NeuronCores.
#
# Strategy (expert-parallel, per sharding hint):
#   - Router (tiny: [T,1024]@[1024,16]) runs on host with jax-CPU, matching the
#     reference's einsum/softmax/top_k numerics so bank selection is identical.
#   - Host dispatches tokens to banks (the "all-to-all"): per-bank gather of the
#     selected tokens, padded to a uniform capacity C, transposed to feature-major.
#   - Each core owns 2 of the 16 banks and runs the two-layer FFN for its banks'
#     tokens: HT = relu(W1^T X^T + b1), YT = W2^T HT + b2, all feature-major so
#     weights are consumed in their natural [K, M] layout as matmul lhsT and no
#     on-device transposes are needed.
#   - Host combines: out[t] = sum_e gate[t,e] * Y_e[t].
#
# Matmul dtype: float32r (full-rate fp32-storage matmul, ~1e-4 relative error).
# Set BANKED_DTYPE=bf16 to halve weight DMA at ~2e-3 error.

import os
import sys
import types

import numpy as np

D_MODEL = 1024
D_HIDDEN = 4096
NUM_BANKS = 16
NUM_SELECTED = 2
N_CORES = 8
P = 128
KD = D_MODEL // P    # 8  contraction subtiles for fc1
MH = D_HIDDEN // P   # 32 hidden subtiles (fc1 out / fc2 contraction)
MO = D_MODEL // P    # 8  output subtiles for fc2

_PROFILE = bool(int(os.environ.get("BANKED_PROFILE", "0")))
_DTYPE = os.environ.get("BANKED_DTYPE", "f32r")  # "f32r" | "bf16" | "f32"

last_exec_time_ns = None
last_results = None


def _install_ntff_hook():
    """bass_utils' axon trace path imports antenv.axon_hooks, which this image
    lacks; shim it and register the ctypes-based NTFF hook."""
    if "antenv.axon_hooks" in sys.modules:
        return
    mod = types.ModuleType("antenv.axon_hooks")
    mod._hook = None
    mod.set_axon_ntff_profile_hook = lambda h: setattr(mod, "_hook", h)
    mod.get_axon_ntff_profile_hook = lambda: mod._hook
    sys.modules["antenv.axon_hooks"] = mod
    try:
        from trn_agent_boot.trn_boot import _ntff_profile_via_ctypes

        mod.set_axon_ntff_profile_hook(
            _ntff_profile_via_ctypes("/opt/axon/libaxon_pjrt.so")
        )
    except Exception as e:  # profiling is best-effort
        print("ntff hook setup failed:", e)


def _router(tensor_f32, Wr, br):
    """Return (topv, topi) exactly as the reference computes them (jax on CPU)."""
    try:
        import jax
        import jax.numpy as jnp

        cpu = jax.devices("cpu")[0]
        with jax.default_device(cpu):
            t = jax.device_put(jnp.asarray(tensor_f32), cpu)
            w = jax.device_put(jnp.asarray(Wr), cpu)
            b = jax.device_put(jnp.asarray(br), cpu)
            logits = jnp.einsum("bsd,de->bse", t, w) + b
            probs = jax.nn.softmax(logits, axis=-1)
            topv, topi = jax.lax.top_k(probs, NUM_SELECTED)
        return np.asarray(topv), np.asarray(topi)
    except Exception:
        # numpy fallback replicating jax semantics (stable ties by lower index)
        logits = (
            tensor_f32.reshape(-1, D_MODEL) @ np.asarray(Wr, np.float32)
        ) + np.asarray(br, np.float32)
        logits = logits.reshape(tensor_f32.shape[0], tensor_f32.shape[1], NUM_BANKS)
        m = logits.max(axis=-1, keepdims=True)
        e = np.exp(logits - m)
        probs = e / e.sum(axis=-1, keepdims=True)
        order = np.argsort(-probs, axis=-1, kind="stable")
        topi = order[..., :NUM_SELECTED]
        topv = np.take_along_axis(probs, topi, axis=-1)
        return topv.astype(np.float32), topi.astype(np.int32)


def _chunks_for(C):
    """Split C into free-dim chunks <= 512, preferring every chunk >= 256
    (float32r runs full-rate only at free dim >= 256)."""
    out = []
    t0 = 0
    n = (C + 511) // 512
    base = C // n
    rem = C - base * n
    for i in range(n):
        tn = base + (1 if i < rem else 0)
        out.append((t0, tn))
        t0 += tn
    return out


def _build_program(C, chunks, dtype_tag):
    import concourse.mybir as mybir
    import concourse.tile as tile
    from concourse import bacc

    if dtype_tag == "bf16":
        wdt = mybir.dt.bfloat16
    elif dtype_tag == "f32":
        wdt = mybir.dt.float32
    else:
        # float32r end-to-end: walrus requires f32r matmul operands to be
        # produced as f32r, so the dram tensors and sbuf tiles are all f32r
        # (np-side arrays stay float32 — same 4-byte storage).
        wdt = mybir.dt.float32r

    f32 = mybir.dt.float32
    nc = bacc.Bacc("TRN2", target_bir_lowering=False, debug=False, num_devices=N_CORES)

    xt = nc.dram_tensor("xt", [2, KD, P, C], wdt, kind="ExternalInput").ap()
    w1 = nc.dram_tensor("w1", [2, MH, P, KD, P], wdt, kind="ExternalInput").ap()
    b1 = nc.dram_tensor("b1", [2, P, MH], f32, kind="ExternalInput").ap()
    w2 = nc.dram_tensor("w2", [2, MO, P, MH, P], wdt, kind="ExternalInput").ap()
    b2 = nc.dram_tensor("b2", [2, P, MO], f32, kind="ExternalInput").ap()
    yt = nc.dram_tensor("yt", [2, MO, P, C], f32, kind="ExternalOutput").ap()

    Relu = mybir.ActivationFunctionType.Relu
    Ident = mybir.ActivationFunctionType.Identity

    with tile.TileContext(nc) as tc:
        with (
            tc.tile_pool(name="xp", bufs=1) as xp,
            tc.tile_pool(name="bp", bufs=1) as bp,
            tc.tile_pool(name="w1p", bufs=3) as w1p,
            tc.tile_pool(name="w2p", bufs=2) as w2p,
            tc.tile_pool(name="htp", bufs=1) as htp,
            tc.tile_pool(name="yp", bufs=4) as yp,
            tc.tile_pool(name="ps1", bufs=4, space="PSUM") as ps1,
            tc.tile_pool(name="ps2", bufs=4, space="PSUM") as ps2,
        ):
            xsb = xp.tile([P, 2 * KD, C], wdt)
            nc.sync.dma_start(xsb[:], xt.rearrange("e k p c -> p (e k) c"))
            b1sb = bp.tile([P, 2, MH], f32, tag="b1")
            nc.sync.dma_start(b1sb[:], b1.rearrange("e p m -> p e m"))
            b2sb = bp.tile([P, 2, MO], f32, tag="b2")
            nc.sync.dma_start(b2sb[:], b2.rearrange("e p m -> p e m"))

            tcmax = max(tn for _, tn in chunks)
            for e in range(2):
                ht = htp.tile([P, MH, C], wdt, tag="ht")
                for hm in range(MH):
                    w1sb = w1p.tile([P, KD, P], wdt, tag="w1")
                    nc.sync.dma_start(w1sb[:], w1[e, hm])
                    for (t0, tn) in chunks:
                        ps = ps1.tile([P, tcmax], f32, tag="ps1")
                        for k in range(KD):
                            nc.tensor.matmul(
                                ps[:, :tn],
                                w1sb[:, k],
                                xsb[:, e * KD + k, t0 : t0 + tn],
                                start=(k == 0),
                                stop=(k == KD - 1),
                            )
                        nc.scalar.activation(
                            ht[:, hm, t0 : t0 + tn],
                            ps[:, :tn],
                            Relu,
                            bias=b1sb[:, e, hm : hm + 1],
                        )
                for mo in range(MO):
                    w2sb = w2p.tile([P, MH, P], wdt, tag="w2")
                    nc.sync.dma_start(w2sb[:], w2[e, mo])
                    for (t0, tn) in chunks:
                        ps = ps2.tile([P, tcmax], f32, tag="ps2")
                        for k2 in range(MH):
                            nc.tensor.matmul(
                                ps[:, :tn],
                                w2sb[:, k2],
                                ht[:, k2, t0 : t0 + tn],
                                start=(k2 == 0),
                                stop=(k2 == MH - 1),
                            )
                        ysb = yp.tile([P, tcmax], f32, tag="y")
                        nc.scalar.activation(
                            ysb[:, :tn],
                            ps[:, :tn],
                            Ident,
                            bias=b2sb[:, e, mo : mo + 1],
                        )
                        nc.sync.dma_start(yt[e, mo, :, t0 : t0 + tn], ysb[:, :tn])

    nc.compile()
    return nc


def kernel(tensor, Wr, br, W1, b1, W2, b2):
    global last_exec_time_ns, last_results
    from concourse import bass_utils

    t_np = np.asarray(tensor, np.float32)
    B, S, _ = t_np.shape
    T = B * S
    x = np.ascontiguousarray(t_np.reshape(T, D_MODEL))

    topv, topi = _router(t_np, np.asarray(Wr, np.float32), np.asarray(br, np.float32))
    topv = topv.reshape(T, NUM_SELECTED)
    topi = topi.reshape(T, NUM_SELECTED)

    # Per-bank token index lists + gates
    idx = []
    gates = []
    for e in range(NUM_BANKS):
        sel = np.nonzero((topi == e).any(axis=1))[0]
        idx.append(sel)
        g = np.where(topi[sel, 0] == e, topv[sel, 0], topv[sel, 1])
        gates.append(g.astype(np.float32))

    counts = np.array([len(i) for i in idx])
    C = int(counts.max())
    C = max(C, P)
    C = (C + 1) & ~1  # even, so chunks split evenly
    chunks = _chunks_for(C)

    # Feature-major gathered tokens: xt_all[e, k, p, t] = x[idx[e][t], k*P+p]
    if _DTYPE == "bf16":
        import ml_dtypes

        np_wdt = ml_dtypes.bfloat16
    else:
        np_wdt = np.float32

    xt_all = np.zeros((NUM_BANKS, KD, P, C), dtype=np_wdt)
    for e in range(NUM_BANKS):
        n_e = counts[e]
        if n_e:
            xe_t = x[idx[e]].T.astype(np_wdt)  # [D_MODEL, n_e]
            xt_all[e, :, :, :n_e] = xe_t.reshape(KD, P, n_e)

    W1_np = np.asarray(W1, np.float32)
    W2_np = np.asarray(W2, np.float32)
    w1d = np.ascontiguousarray(
        W1_np.reshape(NUM_BANKS, KD, P, MH, P).transpose(0, 3, 2, 1, 4).astype(np_wdt)
    )
    w2d = np.ascontiguousarray(
        W2_np.reshape(NUM_BANKS, MH, P, MO, P).transpose(0, 3, 2, 1, 4).astype(np_wdt)
    )
    b1d = np.ascontiguousarray(
        np.asarray(b1, np.float32).reshape(NUM_BANKS, MH, P).transpose(0, 2, 1)
    )
    b2d = np.ascontiguousarray(
        np.asarray(b2, np.float32).reshape(NUM_BANKS, MO, P).transpose(0, 2, 1)
    )

    nc = _build_program(C, chunks, _DTYPE)

    in_maps = []
    for c in range(N_CORES):
        lo, hi = 2 * c, 2 * c + 2
        in_maps.append(
            {
                "xt": xt_all[lo:hi],
                "w1": w1d[lo:hi],
                "b1": b1d[lo:hi],
                "w2": w2d[lo:hi],
                "b2": b2d[lo:hi],
            }
        )

    if _PROFILE:
        _install_ntff_hook()
    res = bass_utils.run_bass_kernel_spmd(
        nc, in_maps, core_ids=list(range(N_CORES)), trace=_PROFILE
    )
    last_exec_time_ns = res.exec_time_ns
    last_results = res

    out = np.zeros((T, D_MODEL), dtype=np.float32)
    for c in range(N_CORES):
        ytc = res.results[c]["yt"]  # [2, MO, P, C] f32
        for j in range(2):
            e = 2 * c + j
            n_e = counts[e]
            if n_e:
                ye = ytc[j].reshape(D_MODEL, C)[:, :n_e]  # [D_MODEL, n_e]
                out[idx[e]] += gates[e][:, None] * ye.T
    return out.reshape(B, S, D_MODEL)


# revision 8
# speedup vs baseline: 1.1428x; 1.1428x over previous
# Banked (MoE top-2) feedforward on 8 TRN2 NeuronCores.
#
# Strategy (expert-parallel, per sharding hint):
#   - Router (tiny: [T,1024]@[1024,16]) runs on host with jax-CPU, matching the
#     reference's einsum/softmax/top_k numerics so bank selection is identical.
#   - Host dispatches tokens to banks (the "all-to-all"): per-bank gather of the
#     selected tokens, padded to a uniform capacity C, transposed to feature-major.
#   - Each core owns 2 of the 16 banks and runs the two-layer FFN for its banks'
#     tokens: HT = relu(W1^T X^T + b1), YT = W2^T HT + b2, all feature-major so
#     weights are consumed in their natural [K, M] layout as matmul lhsT and no
#     on-device transposes are needed.
#   - Host combines: out[t] = sum_e gate[t,e] * Y_e[t].
#
# Matmul dtype: float32r (full-rate fp32-storage matmul, ~1e-4 relative error).
# Set BANKED_DTYPE=bf16 to halve weight DMA at ~2e-3 error.

import os
import sys
import types

import numpy as np

D_MODEL = 1024
D_HIDDEN = 4096
NUM_BANKS = 16
NUM_SELECTED = 2
N_CORES = 8
P = 128
KD = D_MODEL // P    # 8  contraction subtiles for fc1
MH = D_HIDDEN // P   # 32 hidden subtiles (fc1 out / fc2 contraction)
MO = D_MODEL // P    # 8  output subtiles for fc2

_PROFILE = bool(int(os.environ.get("BANKED_PROFILE", "0")))
_DTYPE = os.environ.get("BANKED_DTYPE", "f32r")  # "f32r" | "bf16" | "f32"

last_exec_time_ns = None
last_results = None


def _install_ntff_hook():
    """bass_utils' axon trace path imports antenv.axon_hooks, which this image
    lacks; shim it and register the ctypes-based NTFF hook."""
    if "antenv.axon_hooks" in sys.modules:
        return
    mod = types.ModuleType("antenv.axon_hooks")
    mod._hook = None
    mod.set_axon_ntff_profile_hook = lambda h: setattr(mod, "_hook", h)
    mod.get_axon_ntff_profile_hook = lambda: mod._hook
    sys.modules["antenv.axon_hooks"] = mod
    try:
        from trn_agent_boot.trn_boot import _ntff_profile_via_ctypes

        mod.set_axon_ntff_profile_hook(
            _ntff_profile_via_ctypes("/opt/axon/libaxon_pjrt.so")
        )
    except Exception as e:  # profiling is best-effort
        print("ntff hook setup failed:", e)


def _router(tensor_f32, Wr, br):
    """Return (topv, topi) exactly as the reference computes them (jax on CPU)."""
    try:
        import jax
        import jax.numpy as jnp

        cpu = jax.devices("cpu")[0]
        with jax.default_device(cpu):
            t = jax.device_put(jnp.asarray(tensor_f32), cpu)
            w = jax.device_put(jnp.asarray(Wr), cpu)
            b = jax.device_put(jnp.asarray(br), cpu)
            logits = jnp.einsum("bsd,de->bse", t, w) + b
            probs = jax.nn.softmax(logits, axis=-1)
            topv, topi = jax.lax.top_k(probs, NUM_SELECTED)
        return np.asarray(topv), np.asarray(topi)
    except Exception:
        # numpy fallback replicating jax semantics (stable ties by lower index)
        logits = (
            tensor_f32.reshape(-1, D_MODEL) @ np.asarray(Wr, np.float32)
        ) + np.asarray(br, np.float32)
        logits = logits.reshape(tensor_f32.shape[0], tensor_f32.shape[1], NUM_BANKS)
        m = logits.max(axis=-1, keepdims=True)
        e = np.exp(logits - m)
        probs = e / e.sum(axis=-1, keepdims=True)
        order = np.argsort(-probs, axis=-1, kind="stable")
        topi = order[..., :NUM_SELECTED]
        topv = np.take_along_axis(probs, topi, axis=-1)
        return topv.astype(np.float32), topi.astype(np.int32)


def _chunks_for(C):
    """Split C into free-dim chunks <= 512, preferring every chunk >= 256
    (float32r runs full-rate only at free dim >= 256)."""
    out = []
    t0 = 0
    n = (C + 511) // 512
    base = C // n
    rem = C - base * n
    for i in range(n):
        tn = base + (1 if i < rem else 0)
        out.append((t0, tn))
        t0 += tn
    return out


def _build_program(C, chunks, dtype_tag):
    import concourse.mybir as mybir
    import concourse.tile as tile
    from concourse import bacc

    if dtype_tag == "bf16":
        wdt = mybir.dt.bfloat16
    elif dtype_tag == "f32":
        wdt = mybir.dt.float32
    else:
        # float32r end-to-end: walrus requires f32r matmul operands to be
        # produced as f32r, so the dram tensors and sbuf tiles are all f32r
        # (np-side arrays stay float32 — same 4-byte storage).
        wdt = mybir.dt.float32r

    f32 = mybir.dt.float32
    nc = bacc.Bacc("TRN2", target_bir_lowering=False, debug=False, num_devices=N_CORES)

    xt = nc.dram_tensor("xt", [2, KD, P, C], wdt, kind="ExternalInput").ap()
    w1 = nc.dram_tensor("w1", [2, MH, P, KD, P], wdt, kind="ExternalInput").ap()
    b1 = nc.dram_tensor("b1", [2, P, MH], f32, kind="ExternalInput").ap()
    w2 = nc.dram_tensor("w2", [2, MO, P, MH, P], wdt, kind="ExternalInput").ap()
    b2 = nc.dram_tensor("b2", [2, P, MO], f32, kind="ExternalInput").ap()
    yt = nc.dram_tensor("yt", [2, MO, P, C], f32, kind="ExternalOutput").ap()

    Add = mybir.AluOpType.add
    Max = mybir.AluOpType.max

    with tile.TileContext(nc) as tc:
        with (
            tc.tile_pool(name="xp", bufs=1) as xp,
            tc.tile_pool(name="bp", bufs=1) as bp,
            tc.tile_pool(name="w1p", bufs=4) as w1p,
            tc.tile_pool(name="w2p", bufs=3) as w2p,
            tc.tile_pool(name="htp", bufs=1) as htp,
            tc.tile_pool(name="yp", bufs=4) as yp,
            tc.tile_pool(name="ps1", bufs=4, space="PSUM") as ps1,
            tc.tile_pool(name="ps2", bufs=4, space="PSUM") as ps2,
        ):
            # per-(bank, k) x tiles: contiguous DMAs, fine-grained deps so
            # the first matmuls start as soon as their slice lands.
            # Issue order: bank0's first weight tile + bank0 x first, so the
            # first matmuls aren't stuck behind the whole x/bias preload.
            w1sb0 = w1p.tile([P, KD, P], wdt, tag="w1")
            nc.sync.dma_start(w1sb0[:], w1[0, 0])
            xsb = {}
            for e in range(2):
                for k in range(KD):
                    t = xp.tile([P, C], wdt, tag=f"x_{e}_{k}")
                    nc.sync.dma_start(t[:], xt[e, k])
                    xsb[(e, k)] = t
                if e == 0:
                    b1sb = bp.tile([P, 2, MH], f32, tag="b1")
                    nc.sync.dma_start(b1sb[:], b1.rearrange("e p m -> p e m"))
                    b2sb = bp.tile([P, 2, MO], f32, tag="b2")
                    nc.sync.dma_start(b2sb[:], b2.rearrange("e p m -> p e m"))

            tcmax = max(tn for _, tn in chunks)
            for e in range(2):
                ht = htp.tile([P, MH, C], wdt, tag="ht")
                for hm in range(MH):
                    if e == 0 and hm == 0:
                        w1sb = w1sb0
                    else:
                        w1sb = w1p.tile([P, KD, P], wdt, tag="w1")
                        nc.sync.dma_start(w1sb[:], w1[e, hm])
                    # k outer, chunks inner: one weight load feeds all chunks
                    pss = [ps1.tile([P, tcmax], f32, tag="ps1", name=f"ps1_{ci}") for ci in range(len(chunks))]
                    for k in range(KD):
                        for ci, (t0, tn) in enumerate(chunks):
                            nc.tensor.matmul(
                                pss[ci][:, :tn],
                                w1sb[:, k],
                                xsb[(e, k)][:, t0 : t0 + tn],
                                start=(k == 0),
                                stop=(k == KD - 1),
                            )
                    for ci, (t0, tn) in enumerate(chunks):
                        # relu(psum + b1) on DVE (ACT's per-op overhead is huge)
                        nc.vector.tensor_scalar(
                            ht[:, hm, t0 : t0 + tn],
                            pss[ci][:, :tn],
                            b1sb[:, e, hm : hm + 1],
                            0.0,
                            Add,
                            Max,
                        )
                for mo in range(MO):
                    w2sb = w2p.tile([P, MH, P], wdt, tag="w2")
                    nc.sync.dma_start(w2sb[:], w2[e, mo])
                    pss = [ps2.tile([P, tcmax], f32, tag="ps2", name=f"ps2_{ci}") for ci in range(len(chunks))]
                    for k2 in range(MH):
                        for ci, (t0, tn) in enumerate(chunks):
                            nc.tensor.matmul(
                                pss[ci][:, :tn],
                                w2sb[:, k2],
                                ht[:, k2, t0 : t0 + tn],
                                start=(k2 == 0),
                                stop=(k2 == MH - 1),
                            )
                    for ci, (t0, tn) in enumerate(chunks):
                        ysb = yp.tile([P, tcmax], f32, tag="y")
                        nc.vector.tensor_scalar_add(
                            ysb[:, :tn],
                            pss[ci][:, :tn],
                            b2sb[:, e, mo : mo + 1],
                        )
                        nc.sync.dma_start(yt[e, mo, :, t0 : t0 + tn], ysb[:, :tn])

    nc.compile()
    return nc


def kernel(tensor, Wr, br, W1, b1, W2, b2):
    global last_exec_time_ns, last_results
    from concourse import bass_utils

    t_np = np.asarray(tensor, np.float32)
    B, S, _ = t_np.shape
    T = B * S
    x = np.ascontiguousarray(t_np.reshape(T, D_MODEL))

    topv, topi = _router(t_np, np.asarray(Wr, np.float32), np.asarray(br, np.float32))
    topv = topv.reshape(T, NUM_SELECTED)
    topi = topi.reshape(T, NUM_SELECTED)

    # Per-bank token index lists + gates
    idx = []
    gates = []
    for e in range(NUM_BANKS):
        sel = np.nonzero((topi == e).any(axis=1))[0]
        idx.append(sel)
        g = np.where(topi[sel, 0] == e, topv[sel, 0], topv[sel, 1])
        gates.append(g.astype(np.float32))

    counts = np.array([len(i) for i in idx])
    C = int(counts.max())
    C = max(C, P)
    C = (C + 1) & ~1  # even, so chunks split evenly
    chunks = _chunks_for(C)

    # Feature-major gathered tokens: xt_all[e, k, p, t] = x[idx[e][t], k*P+p]
    if _DTYPE == "bf16":
        import ml_dtypes

        np_wdt = ml_dtypes.bfloat16
    else:
        np_wdt = np.float32

    xt_all = np.zeros((NUM_BANKS, KD, P, C), dtype=np_wdt)
    for e in range(NUM_BANKS):
        n_e = counts[e]
        if n_e:
            xe_t = x[idx[e]].T.astype(np_wdt)  # [D_MODEL, n_e]
            xt_all[e, :, :, :n_e] = xe_t.reshape(KD, P, n_e)

    W1_np = np.asarray(W1, np.float32)
    W2_np = np.asarray(W2, np.float32)
    w1d = np.ascontiguousarray(
        W1_np.reshape(NUM_BANKS, KD, P, MH, P).transpose(0, 3, 2, 1, 4).astype(np_wdt)
    )
    w2d = np.ascontiguousarray(
        W2_np.reshape(NUM_BANKS, MH, P, MO, P).transpose(0, 3, 2, 1, 4).astype(np_wdt)
    )
    b1d = np.ascontiguousarray(
        np.asarray(b1, np.float32).reshape(NUM_BANKS, MH, P).transpose(0, 2, 1)
    )
    b2d = np.ascontiguousarray(
        np.asarray(b2, np.float32).reshape(NUM_BANKS, MO, P).transpose(0, 2, 1)
    )

    nc = _build_program(C, chunks, _DTYPE)

    in_maps = []
    for c in range(N_CORES):
        lo, hi = 2 * c, 2 * c + 2
        in_maps.append(
            {
                "xt": xt_all[lo:hi],
                "w1": w1d[lo:hi],
                "b1": b1d[lo:hi],
                "w2": w2d[lo:hi],
                "b2": b2d[lo:hi],
            }
        )

    if _PROFILE:
        _install_ntff_hook()
    res = bass_utils.run_bass_kernel_spmd(
        nc, in_maps, core_ids=list(range(N_CORES)), trace=_PROFILE
    )
    last_exec_time_ns = res.exec_time_ns
    last_results = res

    out = np.zeros((T, D_MODEL), dtype=np.float32)
    for c in range(N_CORES):
        ytc = res.results[c]["yt"]  # [2, MO, P, C] f32
        for j in range(2):
            e = 2 * c + j
            n_e = counts[e]
            if n_e:
                ye = ytc[j].reshape(D_MODEL, C)[:, :n_e]  # [D_MODEL, n_e]
                out[idx[e]] += gates[e][:, None] * ye.T
    return out.reshape(B, S, D_MODEL)


# revision 11
# speedup vs baseline: 1.3055x; 1.1423x over previous
# Banked (MoE top-2) feedforward on 8 TRN2 NeuronCores.
#
# Strategy (expert-parallel, per sharding hint):
#   - Router (tiny: [T,1024]@[1024,16]) runs on host with jax-CPU, matching the
#     reference's einsum/softmax/top_k numerics so bank selection is identical.
#   - Host dispatches tokens to banks (the "all-to-all"): per-bank gather of the
#     selected tokens, padded to a uniform capacity C, transposed to feature-major.
#   - Each core owns 2 of the 16 banks and runs the two-layer FFN for its banks'
#     tokens: HT = relu(W1^T X^T + b1), YT = W2^T HT + b2, all feature-major so
#     weights are consumed in their natural [K, M] layout as matmul lhsT and no
#     on-device transposes are needed.
#   - Host combines: out[t] = sum_e gate[t,e] * Y_e[t].
#
# Matmul dtype: float32r (full-rate fp32-storage matmul, ~1e-4 relative error).
# Set BANKED_DTYPE=bf16 to halve weight DMA at ~2e-3 error.

import os
import sys
import types

import numpy as np

D_MODEL = 1024
D_HIDDEN = 4096
NUM_BANKS = 16
NUM_SELECTED = 2
N_CORES = 8
P = 128
KD = D_MODEL // P    # 8  contraction subtiles for fc1
MH = D_HIDDEN // P   # 32 hidden subtiles (fc1 out / fc2 contraction)
MO = D_MODEL // P    # 8  output subtiles for fc2

_PROFILE = bool(int(os.environ.get("BANKED_PROFILE", "0")))
_DTYPE = os.environ.get("BANKED_DTYPE", "f32r")  # "f32r" | "bf16" | "f32"

last_exec_time_ns = None
last_results = None


def _install_ntff_hook():
    """bass_utils' axon trace path imports antenv.axon_hooks, which this image
    lacks; shim it and register the ctypes-based NTFF hook."""
    if "antenv.axon_hooks" in sys.modules:
        return
    mod = types.ModuleType("antenv.axon_hooks")
    mod._hook = None
    mod.set_axon_ntff_profile_hook = lambda h: setattr(mod, "_hook", h)
    mod.get_axon_ntff_profile_hook = lambda: mod._hook
    sys.modules["antenv.axon_hooks"] = mod
    try:
        from trn_agent_boot.trn_boot import _ntff_profile_via_ctypes

        mod.set_axon_ntff_profile_hook(
            _ntff_profile_via_ctypes("/opt/axon/libaxon_pjrt.so")
        )
    except Exception as e:  # profiling is best-effort
        print("ntff hook setup failed:", e)


def _router(tensor_f32, Wr, br):
    """Return (topv, topi) exactly as the reference computes them (jax on CPU)."""
    try:
        import jax
        import jax.numpy as jnp

        cpu = jax.devices("cpu")[0]
        with jax.default_device(cpu):
            t = jax.device_put(jnp.asarray(tensor_f32), cpu)
            w = jax.device_put(jnp.asarray(Wr), cpu)
            b = jax.device_put(jnp.asarray(br), cpu)
            logits = jnp.einsum("bsd,de->bse", t, w) + b
            probs = jax.nn.softmax(logits, axis=-1)
            topv, topi = jax.lax.top_k(probs, NUM_SELECTED)
        return np.asarray(topv), np.asarray(topi)
    except Exception:
        # numpy fallback replicating jax semantics (stable ties by lower index)
        logits = (
            tensor_f32.reshape(-1, D_MODEL) @ np.asarray(Wr, np.float32)
        ) + np.asarray(br, np.float32)
        logits = logits.reshape(tensor_f32.shape[0], tensor_f32.shape[1], NUM_BANKS)
        m = logits.max(axis=-1, keepdims=True)
        e = np.exp(logits - m)
        probs = e / e.sum(axis=-1, keepdims=True)
        order = np.argsort(-probs, axis=-1, kind="stable")
        topi = order[..., :NUM_SELECTED]
        topv = np.take_along_axis(probs, topi, axis=-1)
        return topv.astype(np.float32), topi.astype(np.int32)


def _chunks_for(C):
    """Split C into free-dim chunks <= 512, preferring every chunk >= 256
    (float32r runs full-rate only at free dim >= 256)."""
    out = []
    t0 = 0
    n = (C + 511) // 512
    base = C // n
    rem = C - base * n
    for i in range(n):
        tn = base + (1 if i < rem else 0)
        out.append((t0, tn))
        t0 += tn
    return out


def _build_program(C, chunks, dtype_tag):
    import concourse.mybir as mybir
    import concourse.tile as tile
    from concourse import bacc

    if dtype_tag == "bf16":
        wdt = mybir.dt.bfloat16
    elif dtype_tag == "f32":
        wdt = mybir.dt.float32
    else:
        # float32r end-to-end: walrus requires f32r matmul operands to be
        # produced as f32r, so the dram tensors and sbuf tiles are all f32r
        # (np-side arrays stay float32 — same 4-byte storage).
        wdt = mybir.dt.float32r

    f32 = mybir.dt.float32
    nc = bacc.Bacc("TRN2", target_bir_lowering=False, debug=False, num_devices=N_CORES)

    xt = nc.dram_tensor("xt", [2, KD, P, C], wdt, kind="ExternalInput").ap()
    w1 = nc.dram_tensor("w1", [2, MH, P, KD, P], wdt, kind="ExternalInput").ap()
    b1 = nc.dram_tensor("b1", [2, P, MH], f32, kind="ExternalInput").ap()
    w2 = nc.dram_tensor("w2", [2, MO, P, MH, P], wdt, kind="ExternalInput").ap()
    b2 = nc.dram_tensor("b2", [2, P, MO], f32, kind="ExternalInput").ap()
    yt = nc.dram_tensor("yt", [2, MO, P, C], f32, kind="ExternalOutput").ap()

    Add = mybir.AluOpType.add
    Max = mybir.AluOpType.max

    with tile.TileContext(nc) as tc:
        with (
            tc.tile_pool(name="xp", bufs=1) as xp,
            tc.tile_pool(name="bp", bufs=1) as bp,
            tc.tile_pool(name="w1p", bufs=4) as w1p,
            tc.tile_pool(name="w2p", bufs=5) as w2p,
            tc.tile_pool(name="htp", bufs=1) as htp,
            tc.tile_pool(name="yp", bufs=4) as yp,
            tc.tile_pool(name="ps1", bufs=4, space="PSUM") as ps1,
            tc.tile_pool(name="ps2", bufs=4, space="PSUM") as ps2,
        ):
            # per-(bank, k) x tiles: contiguous DMAs, fine-grained deps so
            # the first matmuls start as soon as their slice lands.
            # Weights stream on the sync (HWDGE) queue; x and biases go
            # through gpsimd so they never block weight prefetch. Bank 1's
            # x tiles are requested just before bank 1's compute.
            xsb = {}

            def load_x(e):
                for k in range(KD):
                    t = xp.tile([P, C], wdt, tag=f"x_{e}_{k}", name=f"x_{e}_{k}")
                    nc.gpsimd.dma_start(t[:], xt[e, k])
                    xsb[(e, k)] = t

            load_x(0)
            b1sb = bp.tile([P, 2, MH], f32, tag="b1")
            nc.gpsimd.dma_start(b1sb[:], b1.rearrange("e p m -> p e m"))
            b2sb = bp.tile([P, 2, MO], f32, tag="b2")
            nc.gpsimd.dma_start(b2sb[:], b2.rearrange("e p m -> p e m"))

            tcmax = max(tn for _, tn in chunks)
            for e in range(2):
                if e == 1:
                    load_x(1)
                ht = htp.tile([P, MH, C], wdt, tag="ht")
                for hm in range(MH):
                    w1sb = w1p.tile([P, KD, P], wdt, tag="w1")
                    nc.sync.dma_start(w1sb[:], w1[e, hm])
                    # k outer, chunks inner: one weight load feeds all chunks
                    pss = [ps1.tile([P, tcmax], f32, tag="ps1", name=f"ps1_{ci}") for ci in range(len(chunks))]
                    for k in range(KD):
                        for ci, (t0, tn) in enumerate(chunks):
                            nc.tensor.matmul(
                                pss[ci][:, :tn],
                                w1sb[:, k],
                                xsb[(e, k)][:, t0 : t0 + tn],
                                start=(k == 0),
                                stop=(k == KD - 1),
                            )
                    for ci, (t0, tn) in enumerate(chunks):
                        # relu(psum + b1) on DVE (ACT's per-op overhead is huge)
                        nc.vector.tensor_scalar(
                            ht[:, hm, t0 : t0 + tn],
                            pss[ci][:, :tn],
                            b1sb[:, e, hm : hm + 1],
                            0.0,
                            Add,
                            Max,
                        )
                for mo in range(MO):
                    # two half-tiles so the first 16 k2 matmuls can start
                    # after 1MB instead of 2MB of weight DMA
                    MHH = MH // 2
                    w2sbs = []
                    for h in range(2):
                        w2sb = w2p.tile([P, MHH, P], wdt, tag="w2", name=f"w2_{h}")
                        nc.sync.dma_start(w2sb[:], w2[e, mo, :, h * MHH : (h + 1) * MHH])
                        w2sbs.append(w2sb)
                    pss = [ps2.tile([P, tcmax], f32, tag="ps2", name=f"ps2_{ci}") for ci in range(len(chunks))]
                    for k2 in range(MH):
                        for ci, (t0, tn) in enumerate(chunks):
                            nc.tensor.matmul(
                                pss[ci][:, :tn],
                                w2sbs[k2 // MHH][:, k2 % MHH],
                                ht[:, k2, t0 : t0 + tn],
                                start=(k2 == 0),
                                stop=(k2 == MH - 1),
                            )
                    for ci, (t0, tn) in enumerate(chunks):
                        ysb = yp.tile([P, tcmax], f32, tag="y")
                        nc.vector.tensor_scalar_add(
                            ysb[:, :tn],
                            pss[ci][:, :tn],
                            b2sb[:, e, mo : mo + 1],
                        )
                        nc.sync.dma_start(yt[e, mo, :, t0 : t0 + tn], ysb[:, :tn])

    nc.compile()
    return nc


def kernel(tensor, Wr, br, W1, b1, W2, b2):
    global last_exec_time_ns, last_results
    from concourse import bass_utils

    t_np = np.asarray(tensor, np.float32)
    B, S, _ = t_np.shape
    T = B * S
    x = np.ascontiguousarray(t_np.reshape(T, D_MODEL))

    topv, topi = _router(t_np, np.asarray(Wr, np.float32), np.asarray(br, np.float32))
    topv = topv.reshape(T, NUM_SELECTED)
    topi = topi.reshape(T, NUM_SELECTED)

    # Per-bank token index lists + gates
    idx = []
    gates = []
    for e in range(NUM_BANKS):
        sel = np.nonzero((topi == e).any(axis=1))[0]
        idx.append(sel)
        g = np.where(topi[sel, 0] == e, topv[sel, 0], topv[sel, 1])
        gates.append(g.astype(np.float32))

    counts = np.array([len(i) for i in idx])
    C = int(counts.max())
    C = max(C, P)
    C = (C + 1) & ~1  # even, so chunks split evenly
    chunks = _chunks_for(C)

    # Feature-major gathered tokens: xt_all[e, k, p, t] = x[idx[e][t], k*P+p]
    if _DTYPE == "bf16":
        import ml_dtypes

        np_wdt = ml_dtypes.bfloat16
    else:
        np_wdt = np.float32

    xt_all = np.zeros((NUM_BANKS, KD, P, C), dtype=np_wdt)
    for e in range(NUM_BANKS):
        n_e = counts[e]
        if n_e:
            xe_t = x[idx[e]].T.astype(np_wdt)  # [D_MODEL, n_e]
            xt_all[e, :, :, :n_e] = xe_t.reshape(KD, P, n_e)

    W1_np = np.asarray(W1, np.float32)
    W2_np = np.asarray(W2, np.float32)
    w1d = np.ascontiguousarray(
        W1_np.reshape(NUM_BANKS, KD, P, MH, P).transpose(0, 3, 2, 1, 4).astype(np_wdt)
    )
    w2d = np.ascontiguousarray(
        W2_np.reshape(NUM_BANKS, MH, P, MO, P).transpose(0, 3, 2, 1, 4).astype(np_wdt)
    )
    b1d = np.ascontiguousarray(
        np.asarray(b1, np.float32).reshape(NUM_BANKS, MH, P).transpose(0, 2, 1)
    )
    b2d = np.ascontiguousarray(
        np.asarray(b2, np.float32).reshape(NUM_BANKS, MO, P).transpose(0, 2, 1)
    )

    nc = _build_program(C, chunks, _DTYPE)

    in_maps = []
    for c in range(N_CORES):
        lo, hi = 2 * c, 2 * c + 2
        in_maps.append(
            {
                "xt": xt_all[lo:hi],
                "w1": w1d[lo:hi],
                "b1": b1d[lo:hi],
                "w2": w2d[lo:hi],
                "b2": b2d[lo:hi],
            }
        )

    if _PROFILE:
        _install_ntff_hook()
    res = bass_utils.run_bass_kernel_spmd(
        nc, in_maps, core_ids=list(range(N_CORES)), trace=_PROFILE
    )
    last_exec_time_ns = res.exec_time_ns
    last_results = res

    out = np.zeros((T, D_MODEL), dtype=np.float32)
    for c in range(N_CORES):
        ytc = res.results[c]["yt"]  # [2, MO, P, C] f32
        for j in range(2):
            e = 2 * c + j
            n_e = counts[e]
            if n_e:
                ye = ytc[j].reshape(D_MODEL, C)[:, :n_e]  # [D_MODEL, n_e]
                out[idx[e]] += gates[e][:, None] * ye.T
    return out.reshape(B, S, D_MODEL)
